# revision 17
# baseline (speedup 1.0000x reference)
"""Trainium2 Bass kernel for LlamaFlashAttentionMasked (EAGLE3 suffix-block attention).

Sharding: 8 cores = batch(2) x head-group(4). Each core handles 1 batch and
8 q-heads / 2 kv-heads. Per-core partial outputs (after Wo on the core's head
slice) are summed across the 4 head-groups on the host.

v2 design (all-bf16 matmuls, single merged projection+attention stream):
  - dc order: K0, V0, K1, V1, Q0..Q7. Each Q head's attention block is
    emitted immediately after its projection, so softmax/exp latency hides
    under the next head's projection matmuls (PE never stalls).
  - Suffix (EAGLE3 diagonal) work per head is hoisted ahead of the causal
    loop: pool computes qk elementwise products, PE column-sums via a ones
    matmul, scalar exps, pool+DVE build the suffix PV contribution and the
    suffix denominator before the causal half needs them.
  - Causal mask is PRELOADED into PSUM (DVE copy of a [tri|zeros] tile),
    score matmuls accumulate on top (start=False), so exp reads PSUM directly.
  - Queries are trimmed to valid=960 (spec guarantees valid<=960); keys keep
    full 1024 with the causal mask zeroing k>=960 contributions exactly.
  - Wo runs transposed (out[ncol, seq]) so the 960 query trim also cuts its
    streamed columns; host transposes back (free).
"""
import sys
sys.path.insert(0, "/opt/trn_rl_repo")

from contextlib import ExitStack

import numpy as np
import ml_dtypes

import concourse.bacc as bacc
import concourse.tile as tile
import concourse.mybir as mybir
from concourse.bass_utils import run_bass_kernel_spmd
from concourse.masks import make_identity

F32 = mybir.dt.float32
BF16 = mybir.dt.bfloat16
Exp = mybir.ActivationFunctionType.Exp

HIDDEN = 4096
S = 1024
SQ = 960      # valid query length (spec: valid_seq_len <= 960)
NH = 8        # q heads per core
NKV = 2       # kv heads per core
D = 128
LCK = 3
FCH = HIDDEN // 128   # 32 f-chunks
SCALE = 1.0 / np.sqrt(D)
NEG = -1e30

# dc order: K0, V0, K1, V1, Q0..Q7
DC = NH + NKV + NKV


def _build():
    nc = bacc.Bacc("TRN2", target_bir_lowering=False, debug=False, num_devices=8)

    hT_d = nc.dram_tensor("hT", [FCH, 128, S], BF16, kind="ExternalInput").ap()
    w1_d = nc.dram_tensor("w1", [DC, 128, FCH, 128], BF16, kind="ExternalInput").ap()
    cos_d = nc.dram_tensor("cosT", [128, S], F32, kind="ExternalInput").ap()
    sin_d = nc.dram_tensor("sinT", [128, S], F32, kind="ExternalInput").ap()
    ks_d = nc.dram_tensor("ksT", [128, NKV, LCK, SQ], BF16, kind="ExternalInput").ap()
    vs_d = nc.dram_tensor("vsT", [128, NKV, LCK, SQ], BF16, kind="ExternalInput").ap()
    # wo[h, d, ncol_chunk, j]: stationary tiles for the transposed Wo gemm
    wo_d = nc.dram_tensor("wo", [NH, 128, FCH, 128], BF16, kind="ExternalInput").ap()
    # transposed output: [hidden_out, seq<=960]
    out_d = nc.dram_tensor("out", [HIDDEN, SQ], F32, kind="ExternalOutput").ap()

    with tile.TileContext(nc) as tc:
        with ExitStack() as ctx:
            pers = ctx.enter_context(tc.tile_pool(name="pers", bufs=1))
            qt = pers.tile([128, NH, SQ], BF16, tag="qt")       # roped Q^T per head
            kt = pers.tile([128, NKV, S], BF16, tag="kt")       # roped K^T per kv head
            vn = pers.tile([128, 8, NKV, D], BF16, tag="vn")    # V natural [s-part, s-chunk, kv, d]
            ot = pers.tile([128, NH, SQ], BF16, tag="ot")       # normalized attn out (rhs for Wo)
            ones_f = pers.tile([128, 128], BF16, tag="ones")
            nc.vector.memset(ones_f, 1.0)
            # wide causal mask: [k-part, 512]: col c masked (NEG) iff c < k, i.e.
            # triangular in the first 128 cols, zeros after.
            dmask = pers.tile([128, 512], F32, tag="dmask")
            nc.gpsimd.memset(dmask, 0.0)
            nc.gpsimd.affine_select(
                out=dmask[:, 0:128], in_=dmask[:, 0:128],
                compare_op=mybir.AluOpType.is_ge,
                fill=NEG, base=0,
                pattern=[[1, 128]], channel_multiplier=-1,
            )
            ident = pers.tile([128, 128], F32, tag="ident")
            make_identity(nc, ident)

            ksT = pers.tile([128, NKV, LCK, SQ], BF16, tag="ksT")
            vsT = pers.tile([128, NKV, LCK, SQ], BF16, tag="vsT")
            cosT = pers.tile([128, S], F32, tag="cos")
            sinT = pers.tile([128, S], F32, tag="sin")

            # ---------------- merged projection + attention stream -------------
            with ExitStack() as actx:
                pa = actx.enter_context(tc.tile_pool(name="pa", bufs=1))
                wp = actx.enter_context(tc.tile_pool(name="wp", bufs=2))
                rt = actx.enter_context(tc.tile_pool(name="rt", bufs=2))
                sxp = actx.enter_context(tc.tile_pool(name="sxp", bufs=1))
                pap = actx.enter_context(tc.tile_pool(name="pap", bufs=1, space="PSUM"))

                # w DMAs split in 4 so they spread across queues.
                def dma_w(w, dc):
                    for q2 in range(2):
                        nc.sync.dma_start(out=w[:, q2 * 16:(q2 + 1) * 16, :],
                                          in_=w1_d[dc, :, q2 * 16:(q2 + 1) * 16, :])

                hT = pa.tile([128, FCH, S], BF16, tag="hT")
                w0 = wp.tile([128, FCH, 128], BF16, tag="w")
                # issue order gates the start (sync engine ~0.65us per issue):
                # first weight chunk and first hidden chunks come first.
                dma_w(w0, 0)
                for fc in range(2):
                    nc.sync.dma_start(out=hT[:, fc, :], in_=hT_d[fc])
                nc.sync.dma_start(out=cosT, in_=cos_d)
                nc.sync.dma_start(out=sinT, in_=sin_d)
                for fc in range(2, FCH):
                    nc.sync.dma_start(out=hT[:, fc, :], in_=hT_d[fc])
                # ks/vs are partition-major in DRAM: single DMA each
                nc.sync.dma_start(out=ksT, in_=ks_d)
                nc.sync.dma_start(out=vsT, in_=vs_d)

                def proj(dc, width):
                    """project dc-th output chunk; returns list of psum tiles
                    [(ps, qlo, w)] covering cols [0, width)."""
                    if dc == 0:
                        w = w0
                    else:
                        w = wp.tile([128, FCH, 128], BF16, tag="w")
                        dma_w(w, dc)
                    outs = []
                    for sh in range(2):
                        qlo = sh * 512
                        wdt = min(width - qlo, 512)
                        if wdt <= 0:
                            break
                        ps = pap.tile([128, 512], F32, tag="pps", bufs=2)
                        for fc in range(FCH):
                            nc.tensor.matmul(ps[:, 0:wdt], w[:, fc, :],
                                             hT[:, fc, qlo:qlo + wdt],
                                             start=(fc == 0), stop=(fc == FCH - 1))
                        outs.append((ps, qlo, wdt))
                    return outs

                def rope(outs, dest):
                    """dest: [128, >=width] bf16 slice target (qt/kt head slice)"""
                    for ps, qlo, wdt in outs:
                        sl = slice(qlo, qlo + wdt)
                        tcos = rt.tile([128, 512], F32, tag="tcos")
                        nc.vector.tensor_mul(tcos[:, 0:wdt], ps[:, 0:wdt], cosT[:, sl])
                        rot = rt.tile([128, 512], F32, tag="rot")
                        nc.scalar.copy(rot[0:64, 0:wdt], ps[64:128, 0:wdt])
                        nc.scalar.copy(rot[64:128, 0:wdt], ps[0:64, 0:wdt])
                        tsin = rt.tile([128, 512], F32, tag="tsin")
                        nc.vector.tensor_mul(tsin[:, 0:wdt], rot[:, 0:wdt], sinT[:, sl])
                        nc.vector.tensor_add(dest[:, sl], tcos[:, 0:wdt], tsin[:, 0:wdt])

                def vtrans(outs, kv):
                    for ps, qlo, wdt in outs:
                        vstage = rt.tile([128, 512], F32, tag="vstage")
                        nc.vector.tensor_copy(out=vstage[:, 0:wdt], in_=ps[:, 0:wdt])
                        for t4 in range(4):
                            sc = (qlo // 128) + t4
                            tp = pap.tile([128, 512], F32, tag="stp", bufs=2)
                            nc.tensor.transpose(tp[:, 0:128],
                                                vstage[:, t4 * 128:(t4 + 1) * 128], ident)
                            nc.vector.tensor_copy(out=vn[:, sc, kv, :], in_=tp[:, 0:128])

                def scores_block(h, kv, qh):
                    """score matmuls + causal mask + exp for one query half."""
                    qlo = qh * 512
                    qw = min(SQ - qlo, 512)
                    nki = qh * 4 + 4
                    pts = []
                    for ki in range(nki):
                        off = max(0, ki * 128 - qlo)
                        st = pap.tile([128, 512], F32, tag="stp", bufs=2)
                        diag = ki * 128 >= qlo
                        nc.tensor.matmul(
                            st[:, off:qw],
                            kt[:, kv, ki * 128:(ki + 1) * 128],
                            qt[:, h, qlo + off:qlo + qw],
                            start=True, stop=True)
                        if diag:  # causal mask on the 128-wide diagonal block
                            nc.vector.tensor_add(st[:, off:off + 128],
                                                 st[:, off:off + 128],
                                                 dmask[:, 0:128])
                        pt = sxp.tile([128, 512], BF16, tag="pt", bufs=8)
                        nc.scalar.activation(out=pt[:, off:qw], in_=st[:, off:qw],
                                             func=Exp, scale=float(SCALE))
                        pts.append((pt, off))
                    return pts

                def reduce_block(h, kv, qh, pts, acc_s, sts):
                    """denominator + PV matmuls and softmax normalization."""
                    qlo = qh * 512
                    qw = min(SQ - qlo, 512)
                    qsl = slice(qlo, qlo + qw)
                    nki = len(pts)
                    sm_ps = pap.tile([128, 512], F32, tag="red", bufs=2)
                    for i, (pt, off) in enumerate(pts):
                        nc.tensor.matmul(sm_ps[:, off:qw], ones_f, pt[:, off:qw],
                                         start=(i == 0), stop=(i == nki - 1))
                    ot_ps = pap.tile([128, 512], F32, tag="otp", bufs=2)
                    for i, (pt, off) in enumerate(pts):
                        nc.tensor.matmul(ot_ps[:, off:qw], vn[:, i, kv, :],
                                         pt[:, off:qw],
                                         start=(i == 0), stop=(i == nki - 1))
                    stot = rt.tile([128, 512], F32, tag="stot")
                    nc.vector.tensor_add(stot[:, 0:qw], sm_ps[:, 0:qw], sts[:, qsl])
                    nc.vector.reciprocal_approx_fast(out=stot[:, 0:qw],
                                                     in_=stot[:, 0:qw])
                    acc = rt.tile([128, 512], F32, tag="acc")
                    nc.vector.tensor_add(acc[:, 0:qw], ot_ps[:, 0:qw], acc_s[:, qsl])
                    nc.vector.tensor_mul(ot[:, h, qsl], acc[:, 0:qw], stot[:, 0:qw])

                def attention(h):
                    kv = h // (NH // NKV)
                    # ---- suffix: diagonal EAGLE3 blocks (products + exps) ----
                    psts = []
                    for j in range(LCK):
                        tmp = sxp.tile([128, SQ], BF16, tag="tmp", bufs=2)
                        nc.gpsimd.tensor_mul(tmp, qt[:, h, :], ksT[:, kv, j, :])
                        pst = sxp.tile([128, SQ], BF16, tag=f"pst{j}", bufs=1)
                        for qh in range(2):
                            qlo = qh * 512
                            wdt = min(SQ - qlo, 512)
                            sf = pap.tile([128, 512], F32, tag="red", bufs=2)
                            nc.tensor.matmul(sf[:, 0:wdt], ones_f,
                                             tmp[:, qlo:qlo + wdt], start=True, stop=True)
                            nc.scalar.activation(out=pst[:, qlo:qlo + wdt],
                                                 in_=sf[:, 0:wdt], func=Exp,
                                                 scale=float(SCALE))
                        psts.append(pst)
                    # qh0 scores first so their mask-adds lead the DVE queue
                    pts0 = scores_block(h, kv, 0)
                    # ---- suffix accumulation (pool muls + DVE adds) ----
                    acc_s = sxp.tile([128, SQ], F32, tag="accs", bufs=2)
                    nc.gpsimd.tensor_mul(acc_s, psts[0], vsT[:, kv, 0, :])
                    t2 = sxp.tile([128, SQ], BF16, tag="t2", bufs=2)
                    nc.gpsimd.tensor_mul(t2, psts[1], vsT[:, kv, 1, :])
                    nc.vector.tensor_add(acc_s, acc_s, t2)
                    t3 = sxp.tile([128, SQ], BF16, tag="t2", bufs=2)
                    nc.gpsimd.tensor_mul(t3, psts[2], vsT[:, kv, 2, :])
                    nc.vector.tensor_add(acc_s, acc_s, t3)
                    sts = sxp.tile([128, SQ], BF16, tag="stots", bufs=2)
                    nc.vector.tensor_add(sts, psts[0], psts[1])
                    nc.vector.tensor_add(sts, sts, psts[2])
                    # qh0 reduce, then the qh1 half
                    reduce_block(h, kv, 0, pts0, acc_s, sts)
                    pts1 = scores_block(h, kv, 1)
                    reduce_block(h, kv, 1, pts1, acc_s, sts)

                # K0, V0, K1, V1 projections first
                rope(proj(0, S), kt[:, 0, :])
                vtrans(proj(1, S), 0)
                rope(proj(2, S), kt[:, 1, :])
                vtrans(proj(3, S), 1)
                # Q heads; attention(h-1) emitted after proj(h) so its PE work
                # never waits on rope/pool latency, and rope(h) emitted AFTER
                # attention(h-1) so mask-adds lead the in-order DVE queue.
                rope(proj(4, SQ), qt[:, 0, :])
                for h in range(1, NH):
                    outs = proj(4 + h, SQ)
                    # sh0's rope reads gate the psum bank proj(h+1) needs:
                    # emit it before the attention block, sh1 after.
                    rope(outs[:1], qt[:, h, :])
                    attention(h - 1)
                    rope(outs[1:], qt[:, h, :])
                attention(NH - 1)

            # ---------------- output projection (transposed) --------------------
            with ExitStack() as cctx:
                wp2 = cctx.enter_context(tc.tile_pool(name="wp2", bufs=3))
                pcp = cctx.enter_context(tc.tile_pool(name="pcp", bufs=1, space="PSUM"))
                NCG = 8   # ncol groups of 4 chunks (512 outputs each)
                for ncg in range(NCG):
                    wo_t = wp2.tile([128, NH, 4, 128], BF16, tag="wo")
                    for h in range(NH):
                        nc.sync.dma_start(out=wo_t[:, h, :, :],
                                          in_=wo_d[h, :, ncg * 4:(ncg + 1) * 4, :])
                    for nci in range(4):
                        ncol = ncg * 4 + nci
                        for sh in range(2):
                            qlo = sh * 512
                            qw = min(SQ - qlo, 512)
                            fo = pcp.tile([128, 512], F32, tag="fop", bufs=4)
                            for h in range(NH):
                                nc.tensor.matmul(fo[:, 0:qw], wo_t[:, h, nci, :],
                                                 ot[:, h, qlo:qlo + qw],
                                                 start=(h == 0), stop=(h == NH - 1))
                            fo_sb = wp2.tile([128, 512], F32, tag="fosb", bufs=4)
                            if (nci * 2 + sh) % 2 == 0:
                                nc.scalar.copy(fo_sb[:, 0:qw], fo[:, 0:qw])
                            else:
                                nc.vector.tensor_copy(out=fo_sb[:, 0:qw],
                                                      in_=fo[:, 0:qw])
                            nc.sync.dma_start(
                                out=out_d[ncol * 128:(ncol + 1) * 128, qlo:qlo + qw],
                                in_=fo_sb[:, 0:qw])
    nc.compile()
    return nc


_NC = None


def _get_nc():
    global _NC
    if _NC is None:
        _NC = _build()
    return _NC


def kernel(hidden_states, k_suffix, v_suffix, Wq, Wk, Wv, Wo, valid_seq_len):
    B = hidden_states.shape[0]
    valid = int(np.asarray(valid_seq_len))

    # rope tables, transposed to [d, s], sin sign-folded for rotate_half
    inv_freq = 1.0 / (10000.0 ** (np.arange(0, D, 2, dtype=np.float32) / D))
    pos = np.arange(S, dtype=np.float32)
    freqs = pos[:, None] * inv_freq[None, :]
    emb = np.concatenate([freqs, freqs], axis=-1)          # [S, D]
    cosT = np.cos(emb).T.astype(np.float32).copy()         # [D, S]
    sinT = np.sin(emb).T.astype(np.float32).copy()
    sgn = np.where(np.arange(D) < D // 2, -1.0, 1.0).astype(np.float32)
    sinT = sinT * sgn[:, None]

    in_maps = []
    for core in range(8):
        b = core // 4
        hg = core % 4
        qsl = slice(hg * NH * D, (hg + 1) * NH * D)
        kvsl = slice(hg * NKV * D, (hg + 1) * NKV * D)

        hT = np.ascontiguousarray(hidden_states[b].T).reshape(FCH, 128, S)
        # dc order: K0, V0, K1, V1, Q0..Q7
        wk = Wk[:, kvsl]
        wv = Wv[:, kvsl]
        cols = [wk[:, 0:128], wv[:, 0:128], wk[:, 128:256], wv[:, 128:256],
                Wq[:, qsl]]
        w1 = np.concatenate(cols, axis=1)                                    # [4096, 1536]
        w1 = w1.reshape(FCH, 128, DC, 128).transpose(2, 1, 0, 3)             # [DC, 128p, FCH, 128m]
        ks = k_suffix[b, hg * NKV:(hg + 1) * NKV, :, :SQ].transpose(3, 0, 1, 2)  # [128d, NKV, LCK, SQ]
        vs = v_suffix[b, hg * NKV:(hg + 1) * NKV, :, :SQ].transpose(3, 0, 1, 2)
        wo = Wo[hg * NH * D:(hg + 1) * NH * D].reshape(NH, 128, FCH, 128)

        in_maps.append({
            "hT": hT.astype(ml_dtypes.bfloat16),
            "w1": np.ascontiguousarray(w1).astype(ml_dtypes.bfloat16),
            "cosT": cosT,
            "sinT": sinT,
            "ksT": np.ascontiguousarray(ks).astype(ml_dtypes.bfloat16),
            "vsT": np.ascontiguousarray(vs).astype(ml_dtypes.bfloat16),
            "wo": np.ascontiguousarray(wo).astype(ml_dtypes.bfloat16),
        })

    global _LAST_IN_MAPS
    _LAST_IN_MAPS = in_maps
    nc = _get_nc()
    res = run_bass_kernel_spmd(nc, in_maps, core_ids=list(range(8)))

    out = np.zeros((B, S, HIDDEN), dtype=np.float32)
    for core in range(8):
        out[core // 4, :SQ] += res.results[core]["out"].T
    out[:, valid:, :] = 0.0
    return out


if __name__ == "__main__":
    rng = np.random.default_rng(0)
    h = rng.standard_normal((2, S, HIDDEN)).astype(np.float32)
    ks = rng.standard_normal((2, 8, LCK, S, D)).astype(np.float32)
    vs = rng.standard_normal((2, 8, LCK, S, D)).astype(np.float32)
    wq = (rng.standard_normal((HIDDEN, HIDDEN)) * 0.02).astype(np.float32)
    wk = (rng.standard_normal((HIDDEN, 1024)) * 0.02).astype(np.float32)
    wv = (rng.standard_normal((HIDDEN, 1024)) * 0.02).astype(np.float32)
    wo = (rng.standard_normal((HIDDEN, HIDDEN)) * 0.02).astype(np.float32)
    o = kernel(hidden_states=h, k_suffix=ks, v_suffix=vs, Wq=wq, Wk=wk, Wv=wv, Wo=wo,
               valid_seq_len=960)
    print(o.shape, o.dtype, np.abs(o).max())
